# revision 1
# baseline (speedup 1.0000x reference)
"""Distributed attention kernel for 8 Trainium2 NeuronCores.

reference:
    query = features_host @ Q          # [4096, 1024]
    key   = features_guests @ K        # [8192, 1024]
    value = features_guests @ V        # [8192, 1024]
    att   = softmax(query @ key.T / 32, axis=1)
    out   = att @ value                # [4096, 1024]

Sharding: host rows (N=4096) split across 8 cores (512 each). Guest rows
(M=8192) split across 8 cores (1024 each) for the key/value projections.
keyT and value are all-gathered as bf16 in m-half chunks, pipelined so
the attention sweeps stream in behind the collective queue (k0 k1 v0 v1).

Per-core pipeline (bf16 matmuls, fp32 PSUM accumulation):
  guests -> guestsT (bf16 PE transposes) -> keyT halves -> AG k0, AG k1
  value shard halves -> AG v0, AG v1; hostT/qT while AGs fly
  S sweep over m: S = keyT_blk.T @ qT, exp on ScalarE (scale=1/32) into a
    persistent bf16 P matrix; rowsum entirely off the TensorE (VectorE
    accumulates P slices during the sweep, one gpsimd partition_all_reduce
    collapses partitions, PE-transpose + reciprocal for the divisors)
  PV sweep: O[n, 0:1024] accumulated across all m in all 8 PSUM banks,
    launching at the v0 completion semaphore
  divide by rowsum (split across VectorE/ScalarE) and write out.
"""

import sys

for _p in ("/opt/trn_rl_repo", "/root/.axon_site/_ro/trn_rl_repo"):
    if _p not in sys.path:
        sys.path.insert(0, _p)

import numpy as np

N_HOST = 4096
N_GUEST = 8192
DIM = 1024
N_CORES = 8
N_SH = N_HOST // N_CORES      # 512 host rows per core
M_SH = N_GUEST // N_CORES     # 1024 guest rows per core
P = 128

_CACHE = {}


def _build():
    import concourse.bass as bass  # noqa: F401
    import concourse.mybir as mybir
    import concourse.tile as tile
    from concourse.masks import make_identity
    from concourse import bacc
    import concourse.bass_isa as bass_isa
    from concourse.tile import add_dep_helper

    f32 = mybir.dt.float32
    bf16 = mybir.dt.bfloat16
    AF = mybir.ActivationFunctionType

    nc = bacc.Bacc(
        "TRN2",
        target_bir_lowering=False,
        debug=False,
        num_devices=N_CORES,
    )

    host = nc.dram_tensor("host", [N_SH, DIM], f32, kind="ExternalInput").ap()
    guests = nc.dram_tensor("guests", [M_SH, DIM], f32, kind="ExternalInput").ap()
    Qp = nc.dram_tensor("wq", [DIM, DIM], f32, kind="ExternalInput").ap()
    Kp = nc.dram_tensor("wk", [DIM, DIM], f32, kind="ExternalInput").ap()
    Vp = nc.dram_tensor("wv", [DIM, DIM], f32, kind="ExternalInput").ap()
    out = nc.dram_tensor("out", [N_SH, DIM], f32, kind="ExternalOutput").ap()

    RG = [list(range(N_CORES))]
    NMO = N_GUEST // P        # 64 m-chunks of 128

    def AG(in_ap, out_ap):
        nc.gpsimd.collective_compute(
            "AllGather", mybir.AluOpType.bypass, replica_groups=RG,
            ins=[in_ap.opt()], outs=[out_ap.opt()],
        )

    with tile.TileContext(nc) as tc:
        with tc.tile_pool(name="persist", bufs=1) as persist, \
             tc.tile_pool(name="dram", bufs=1, space="DRAM") as dram:

            # ---- DRAM collective buffers (bf16) ----
            k_in = [dram.tile([DIM, 512], bf16, name=f"k_in{h}") for h in range(2)]
            v_in = [dram.tile([512, DIM], bf16, name=f"v_in{h}") for h in range(2)]
            k_out = [dram.tile([N_CORES * DIM, 512], bf16, addr_space="Shared",
                               name=f"k_out{h}") for h in range(2)]
            v_out = [dram.tile([N_CORES * 512, DIM], bf16, addr_space="Shared",
                               name=f"v_out{h}") for h in range(2)]

            # ---- persistent SBUF ----
            qT = persist.tile([P, 8, N_SH], bf16, name="qT")          # [dout_i, dout_o, n]
            Psb = persist.tile([P, NMO, N_SH], bf16, name="Psb")      # [m_i, m_o, n] 8MB
            rs_acc = persist.tile([P, N_SH], f32, name="rs_acc")
            rs_red = persist.tile([P, N_SH], f32, name="rs_red")
            identity = persist.tile([P, P], bf16, name="identity")
            id32 = persist.tile([P, P], f32, name="id32")
            rs_pad = persist.tile([P, N_SH], f32, name="rs_pad")
            rsT = persist.tile([P, 4], f32, name="rsT")
            recip = persist.tile([P, 4], f32, name="recip")

            nc.vector.memset(rs_pad, 0.0)
            make_identity(nc, identity)
            make_identity(nc, id32)

            # ============ pre-flash: projections + AGs ============
            with tc.tile_pool(name="pw", bufs=1) as pw, \
                 tc.tile_pool(name="stage", bufs=4) as stage, \
                 tc.tile_pool(name="ps_tr", bufs=2, space="PSUM") as ps_tr, \
                 tc.tile_pool(name="ps_mm", bufs=4, space="PSUM") as ps_mm:

                guestsT = pw.tile([P, 8, M_SH], bf16, name="guestsT")  # [din_i, din_o, m]

                def load_transpose(src_rows, dst, col):
                    nat = stage.tile([P, DIM], f32, name="nat", tag="stage")
                    nc.sync.dma_start(nat, src_rows)
                    nbf = stage.tile([P, DIM], bf16, name="nbf", tag="stage_bf")
                    nc.vector.tensor_copy(out=nbf, in_=nat)
                    for d in range(8):
                        tps = ps_tr.tile([P, P], bf16, name="tps", tag="tr")
                        nc.tensor.transpose(tps, nbf[:, d * P:(d + 1) * P], identity)
                        nc.vector.tensor_copy(out=dst[:, d, col:col + P], in_=tps)

                for c in range(M_SH // P):
                    load_transpose(guests[c * P:(c + 1) * P, :], guestsT, c * P)
                K_sb = pw.tile([P, 8, DIM], bf16, name="K_sb")
                for c in range(8):
                    w_nat = stage.tile([P, DIM], f32, name="w_nat", tag="stage")
                    nc.sync.dma_start(w_nat, Kp[c * P:(c + 1) * P, :])
                    nc.vector.tensor_copy(out=K_sb[:, c, :], in_=w_nat)

                # keyT shard [dout, m_loc]; m-half chunks feed the k AGs early
                k_loc = pw.tile([P, 8, M_SH], bf16, name="k_loc")
                for mh in range(2):
                    for dc in range(8):
                        mps = ps_mm.tile([P, 512], f32, name="mps", tag="mm")
                        for kc in range(8):
                            nc.tensor.matmul(
                                mps,
                                lhsT=K_sb[:, kc, dc * P:(dc + 1) * P],
                                rhs=guestsT[:, kc, mh * 512:(mh + 1) * 512],
                                start=(kc == 0), stop=(kc == 7),
                            )
                        nc.scalar.copy(out=k_loc[:, dc, mh * 512:(mh + 1) * 512], in_=mps)
                    nc.sync.dma_start(
                        k_in[mh].rearrange("(ko ki) m -> ki ko m", ki=P),
                        k_loc[:, :, mh * 512:(mh + 1) * 512])
                    AG(k_in[mh], k_out[mh])

                # value shard [m_loc, dout] -> one 16MB all-gather
                V_sb = pw.tile([P, 8, DIM], bf16, name="V_sb")
                for c in range(8):
                    w_nat = stage.tile([P, DIM], f32, name="w_nat3", tag="stage")
                    nc.sync.dma_start(w_nat, Vp[c * P:(c + 1) * P, :])
                    nc.vector.tensor_copy(out=V_sb[:, c, :], in_=w_nat)
                v_loc = pw.tile([P, 8, DIM], bf16, name="v_loc")      # [m_i, m_o, dout]
                for mh in range(2):
                    for mc in range(4 * mh, 4 * mh + 4):
                        for dh in range(2):
                            mps = ps_mm.tile([P, 512], f32, name="mps2", tag="mm")
                            for kc in range(8):
                                nc.tensor.matmul(
                                    mps,
                                    lhsT=guestsT[:, kc, mc * P:(mc + 1) * P],
                                    rhs=V_sb[:, kc, dh * 512:(dh + 1) * 512],
                                    start=(kc == 0), stop=(kc == 7),
                                )
                            nc.vector.tensor_copy(
                                out=v_loc[:, mc, dh * 512:(dh + 1) * 512], in_=mps)
                    nc.sync.dma_start(
                        v_in[mh].rearrange("(mo mi) d -> mi mo d", mi=P),
                        v_loc[:, 4 * mh:4 * mh + 4, :])
                    AG(v_in[mh], v_out[mh])

                # hostT + qT (overlaps k AGs)
                Q_sb = pw.tile([P, 8, DIM], bf16, name="Q_sb")
                for c in range(8):
                    w_nat = stage.tile([P, DIM], f32, name="w_nat2", tag="stage")
                    nc.sync.dma_start(w_nat, Qp[c * P:(c + 1) * P, :])
                    nc.vector.tensor_copy(out=Q_sb[:, c, :], in_=w_nat)
                hostT = pw.tile([P, 8, N_SH], bf16, name="hostT")     # [din_i, din_o, n]
                for c in range(N_SH // P):
                    load_transpose(host[c * P:(c + 1) * P, :], hostT, c * P)
                for dc in range(8):
                    qps = ps_mm.tile([P, N_SH], f32, name="qps", tag="mm")
                    for kc in range(8):
                        nc.tensor.matmul(
                            qps,
                            lhsT=Q_sb[:, kc, dc * P:(dc + 1) * P],
                            rhs=hostT[:, kc, :],
                            start=(kc == 0), stop=(kc == 7),
                        )
                    nc.scalar.copy(out=qT[:, dc, :], in_=qps)


            # views of the gathered buffers
            k_out_r = [k_out[h].rearrange("(b o i) m -> b i o m", o=8, i=P)
                       for h in range(2)]
            v_out_r = [v_out[h].rearrange("(b t i) d -> b i t d", t=4, i=P)
                       for h in range(2)]

            # ============ flash: S sweep then PV sweep ============
            with tc.tile_pool(name="kvp", bufs=1) as kvp, \
                 tc.tile_pool(name="outp", bufs=8) as outp:
                with tc.tile_pool(name="ps_st", bufs=3, space="PSUM") as ps_st:
                    t = 0
                    for h in range(2):
                        kts = []
                        for bb in range(8):
                            kT = kvp.tile([P, 8, 512], bf16, name="kT",
                                          tag="kT", bufs=10)
                            nc.sync.dma_start(kT, k_out_r[h][bb])
                            kts.append(kT)
                        for bb in range(8):
                            kT = kts[bb]
                            for j in range(4):
                                mo = bb * 8 + h * 4 + j
                                st = ps_st.tile([P, N_SH], f32, name="st", tag="st")
                                for dc in range(8):
                                    nc.tensor.matmul(
                                        st,
                                        lhsT=kT[:, dc, j * P:(j + 1) * P],
                                        rhs=qT[:, dc, :],
                                        start=(dc == 0), stop=(dc == 7),
                                    )
                                nc.scalar.activation(
                                    Psb[:, mo, :], st, AF.Exp, scale=1.0 / 32.0)
                                # rowsum partials accumulate on the (idle)
                                # vector engine as each P slice lands
                                if t == 0:
                                    nc.vector.tensor_copy(out=rs_acc, in_=Psb[:, mo, :])
                                else:
                                    nc.vector.tensor_add(
                                        out=rs_acc, in0=rs_acc, in1=Psb[:, mo, :])
                                t += 1
                    # reduce the per-partition partials across partitions on
                    # the (idle) gpsimd engine; every partition of rs_red then
                    # holds the full rowsum row
                    nc.gpsimd.partition_all_reduce(
                        rs_red, rs_acc, P, bass_isa.ReduceOp.add)
                    # rowsum [1, n] -> per-partition [n_chunk, 1] via PE transpose
                    nc.vector.tensor_copy(out=rs_pad[0:1, :], in_=rs_red[0:1, :])
                    for c in range(4):
                        tp = ps_st.tile([P, P], f32, name="tp", tag="st")
                        nc.tensor.transpose(tp, rs_pad[:, c * P:(c + 1) * P], id32)
                        nc.vector.tensor_copy(out=rsT[:, c:c + 1], in_=tp[:, 0:1])
                    recip_inst = nc.vector.reciprocal(recip, rsT)

                # ---- PV sweep (full dout, 8 PSUM banks) ----
                with tc.tile_pool(name="ps_o", bufs=8, space="PSUM") as ps_o:
                    o_t = [[ps_o.tile([P, 512], f32, name=f"o_{c}_{hh}", tag="o")
                            for hh in range(2)] for c in range(4)]
                    t = 0
                    first_pv = None
                    for hl in range(2):
                        for bb in range(8):
                            vt = kvp.tile([P, 4, DIM], bf16, name="vt",
                                          tag="vt", bufs=4)
                            nc.sync.dma_start(vt, v_out_r[hl][bb])
                            for j in range(4):
                                mo = bb * 8 + hl * 4 + j
                                for c in range(4):
                                    for hh in range(2):
                                        mm = nc.tensor.matmul(
                                            o_t[c][hh],
                                            lhsT=Psb[:, mo, c * P:(c + 1) * P],
                                            rhs=vt[:, j, hh * 512:(hh + 1) * 512],
                                            start=(t == 0), stop=(t == NMO - 1),
                                        )
                                        if first_pv is None:
                                            first_pv = mm
                                t += 1
                    for c in range(4):
                        for hh in range(2):
                            ot = outp.tile([P, 512], f32, name="ot", tag="ot")
                            # split the final divides across DVE and ACT so the
                            # tail chain halves
                            if hh == 0:
                                nc.vector.tensor_scalar_mul(
                                    ot, o_t[c][hh], recip[:, c:c + 1])
                            else:
                                nc.scalar.mul(ot, o_t[c][hh], recip[:, c:c + 1])
                            nc.sync.dma_start(
                                out[c * P:(c + 1) * P, hh * 512:(hh + 1) * 512], ot)

    nc.compile()
    return nc


def _get_nc():
    if "nc" not in _CACHE:
        _CACHE["nc"] = _build()
    return _CACHE["nc"]


def kernel(features_host, features_guests, Q, K, V):
    from concourse.bass_utils import run_bass_kernel_spmd

    nc = _get_nc()

    fh = np.ascontiguousarray(np.asarray(features_host, dtype=np.float32))
    fg = np.ascontiguousarray(np.asarray(features_guests, dtype=np.float32))
    Qn = np.ascontiguousarray(np.asarray(Q, dtype=np.float32))
    Kn = np.ascontiguousarray(np.asarray(K, dtype=np.float32))
    Vn = np.ascontiguousarray(np.asarray(V, dtype=np.float32))

    in_maps = []
    for c in range(N_CORES):
        in_maps.append({
            "host": np.ascontiguousarray(fh[c * N_SH:(c + 1) * N_SH]),
            "guests": np.ascontiguousarray(fg[c * M_SH:(c + 1) * M_SH]),
            "wq": Qn, "wk": Kn, "wv": Vn,
        })

    res = run_bass_kernel_spmd(nc, in_maps, core_ids=list(range(N_CORES)))
    outs = [np.asarray(res.results[c]["out"]) for c in range(N_CORES)]
    return np.concatenate(outs, axis=0).astype(np.float32)



# revision 2
# speedup vs baseline: 1.7183x; 1.7183x over previous
"""Distributed attention kernel for 8 Trainium2 NeuronCores — zero-collective.

reference:
    query = features_host @ Q          # [4096, 1024]
    key   = features_guests @ K        # [8192, 1024]
    value = features_guests @ V        # [8192, 1024]
    att   = softmax(query @ key.T / 32, axis=1)
    out   = att @ value                # [4096, 1024]

Algebraic restructure so each core needs NO cross-core data:
    S   = query @ key^T = (host @ Q @ K^T) @ guests^T = q2 @ guests^T
    out = softmax(S) @ (guests @ V) = (P_norm @ guests) @ V = T_norm @ V
Host rows (N=4096) are sharded 512/core; guests (all 8192 rows) are
replicated to every core as bf16 in two layouts (transposed for the S
sweep, natural for the T sweep) prepared host-side. K folds into the
query side (q2 = host Q K^T, 2×1.07 GF) and V applies after guest
aggregation (O = T V, 1.07 GF), so the 16 MB keyT / 16 MB value
all-gathers of the collective formulation disappear entirely — along
with the ~100 us entry barrier and ~230 us of serial AG time.

Per-core pipeline (bf16 matmuls, fp32 PSUM accumulation, 20.4 GF):
  A: queryT = Q^T-chunks @ hostT; q2T = K @ queryT        (2.15 GF)
  B: S sweep over 64 m-chunks: S^T = guestsT_blk.T @ q2T, exp on
     ScalarE (scale=1/32) into persistent bf16 P [m, mo, n]; VectorE
     accumulates the rowsum as each chunk lands             (8.6 GF)
  C: T^T sweep: T^T[din, n] += guests_blk.T @ P[:, mo, :] into all 8
     PSUM banks across the 64 m-chunks; gpsimd partition_all_reduce +
     reciprocal of the rowsum run here, off the PE path     (8.6 GF)
  D: normalize T^T by the rowsum row during the PSUM->SBUF copy
     (VectorE tensor_mul), O = T_norm^T-chunks @ V, write out (1.07 GF)
"""

import sys

for _p in ("/opt/trn_rl_repo", "/root/.axon_site/_ro/trn_rl_repo"):
    if _p not in sys.path:
        sys.path.insert(0, _p)

import numpy as np

N_HOST = 4096
N_GUEST = 8192
DIM = 1024
N_CORES = 8
N_SH = N_HOST // N_CORES      # 512 host rows per core
P = 128
NMO = N_GUEST // P            # 64 m-chunks of 128

_CACHE = {}


def _build():
    import concourse.bass as bass  # noqa: F401
    import concourse.mybir as mybir
    import concourse.tile as tile
    from concourse import bacc
    import concourse.bass_isa as bass_isa

    f32 = mybir.dt.float32
    bf16 = mybir.dt.bfloat16
    AF = mybir.ActivationFunctionType

    nc = bacc.Bacc(
        "TRN2",
        target_bir_lowering=False,
        debug=False,
        num_devices=N_CORES,
    )

    # host-prepped layouts (see kernel()):
    #   ht  [128, 8, 512]  = host_slice^T as [din_i, din_o, n]
    #   gt  [64, 128, 8, 128] = guests^T chunks [mo][din_i, din_o, m]
    #   gn  [8192, 1024]   = guests natural (bf16)
    #   wq  [128, 8, 1024] = Q as [din_i, din_o, dout]
    #   wkt [128, 8, 1024] = K^T as [dout_i, dout_o, din]
    #   wv  [128, 8, 1024] = V as [din_i, din_o, dout]
    ht = nc.dram_tensor("ht", [P, 8, N_SH], bf16, kind="ExternalInput").ap()
    gt = nc.dram_tensor("gt", [NMO, P, 8, P], bf16, kind="ExternalInput").ap()
    gn = nc.dram_tensor("gn", [N_GUEST, DIM], bf16, kind="ExternalInput").ap()
    wq = nc.dram_tensor("wq", [P, 8, DIM], bf16, kind="ExternalInput").ap()
    wkt = nc.dram_tensor("wkt", [P, 8, DIM], bf16, kind="ExternalInput").ap()
    wv = nc.dram_tensor("wv", [P, 8, DIM], bf16, kind="ExternalInput").ap()
    out = nc.dram_tensor("out", [N_SH, DIM], f32, kind="ExternalOutput").ap()

    with tile.TileContext(nc) as tc:
        with tc.tile_pool(name="persist", bufs=1) as persist:
            Psb = persist.tile([P, NMO, N_SH], bf16, name="Psb")      # 64KB/part
            qryT = persist.tile([P, 8, N_SH], bf16, name="qryT")
            q2T = persist.tile([P, 8, N_SH], bf16, name="q2T")
            ht_sb = persist.tile([P, 8, N_SH], bf16, name="ht_sb")
            wq_sb = persist.tile([P, 8, DIM], bf16, name="wq_sb")
            wkt_sb = persist.tile([P, 8, DIM], bf16, name="wkt_sb")
            wv_sb = persist.tile([P, 8, DIM], bf16, name="wv_sb")
            tTb = persist.tile([P, 8, N_SH], bf16, name="tTb")
            rs_acc = persist.tile([P, N_SH], f32, name="rs_acc")
            rs_red = persist.tile([P, N_SH], f32, name="rs_red")
            recip_row = persist.tile([P, N_SH], f32, name="recip_row")

            # ---- phase A: loads + query/q2 projections ----
            with tc.tile_pool(name="ps_a", bufs=4, space="PSUM") as ps_a:
                nc.sync.dma_start(ht_sb, ht)
                nc.sync.dma_start(wq_sb, wq)
                nc.sync.dma_start(wkt_sb, wkt)
                for dc in range(8):
                    qp = ps_a.tile([P, N_SH], f32, name="qp", tag="pa")
                    for do in range(8):
                        nc.tensor.matmul(
                            qp,
                            lhsT=wq_sb[:, do, dc * P:(dc + 1) * P],
                            rhs=ht_sb[:, do, :],
                            start=(do == 0), stop=(do == 7),
                        )
                    nc.scalar.copy(out=qryT[:, dc, :], in_=qp)
                for dc in range(8):
                    q2p = ps_a.tile([P, N_SH], f32, name="q2p", tag="pa")
                    for do in range(8):
                        nc.tensor.matmul(
                            q2p,
                            lhsT=wkt_sb[:, do, dc * P:(dc + 1) * P],
                            rhs=qryT[:, do, :],
                            start=(do == 0), stop=(do == 7),
                        )
                    nc.scalar.copy(out=q2T[:, dc, :], in_=q2p)

            # ---- phase B: S sweep (S^T = guestsT.T @ q2T, exp, rowsum) ----
            with tc.tile_pool(name="gtp", bufs=1) as gtp:
                with tc.tile_pool(name="ps_st", bufs=3, space="PSUM") as ps_st:
                    for mo in range(NMO):
                        gt_t = gtp.tile([P, 8, P], bf16, name="gt_t",
                                        tag="gt", bufs=8)
                        nc.sync.dma_start(gt_t, gt[mo])
                        st = ps_st.tile([P, N_SH], f32, name="st", tag="st")
                        for do in range(8):
                            nc.tensor.matmul(
                                st,
                                lhsT=gt_t[:, do, :],
                                rhs=q2T[:, do, :],
                                start=(do == 0), stop=(do == 7),
                            )
                        nc.scalar.activation(
                            Psb[:, mo, :], st, AF.Exp, scale=1.0 / 32.0)
                        if mo == 0:
                            nc.vector.tensor_copy(out=rs_acc, in_=Psb[:, mo, :])
                        else:
                            nc.vector.tensor_add(
                                out=rs_acc, in0=rs_acc, in1=Psb[:, mo, :])

            # wv not needed until phase D; queue its DMA behind the gt stream
            nc.sync.dma_start(wv_sb, wv)

            # rowsum collapse across partitions + reciprocal: runs on
            # gpsimd/DVE during the T sweep, off the PE critical path
            nc.gpsimd.partition_all_reduce(
                rs_red, rs_acc, P, bass_isa.ReduceOp.add)
            nc.vector.reciprocal(recip_row, rs_red)

            # ---- phase C: T^T sweep + normalize; phase D: O = T_norm V ----
            with tc.tile_pool(name="gnp", bufs=1) as gnp:
                with tc.tile_pool(name="ps_t", bufs=8, space="PSUM") as ps_t:
                    tt = [ps_t.tile([P, N_SH], f32, name=f"tt{dc}", tag="tt")
                          for dc in range(8)]
                    for mo in range(NMO):
                        gn_t = gnp.tile([P, DIM], bf16, name="gn_t",
                                        tag="gn", bufs=8)
                        nc.sync.dma_start(gn_t, gn[mo * P:(mo + 1) * P, :])
                        for dc in range(8):
                            nc.tensor.matmul(
                                tt[dc],
                                lhsT=gn_t[:, dc * P:(dc + 1) * P],
                                rhs=Psb[:, mo, :],
                                start=(mo == 0), stop=(mo == NMO - 1),
                            )
                    # normalize during the PSUM->SBUF copy (bf16 out)
                    for dc in range(8):
                        nc.vector.tensor_mul(
                            out=tTb[:, dc, :], in0=tt[dc], in1=recip_row)

                with tc.tile_pool(name="ps_o", bufs=8, space="PSUM") as ps_o, \
                     tc.tile_pool(name="outp", bufs=8) as outp:
                    o_t = [[ps_o.tile([P, N_SH], f32, name=f"o{c}{h}", tag="o")
                            for h in range(2)] for c in range(4)]
                    for dc in range(8):
                        for c in range(4):
                            for h in range(2):
                                nc.tensor.matmul(
                                    o_t[c][h],
                                    lhsT=tTb[:, dc, c * P:(c + 1) * P],
                                    rhs=wv_sb[:, dc, h * N_SH:(h + 1) * N_SH],
                                    start=(dc == 0), stop=(dc == 7),
                                )
                    for c in range(4):
                        for h in range(2):
                            ot = outp.tile([P, N_SH], f32, name="ot", tag="ot")
                            # split tail copies across ACT and DVE
                            if h == 0:
                                nc.scalar.copy(out=ot, in_=o_t[c][h])
                            else:
                                nc.vector.tensor_copy(out=ot, in_=o_t[c][h])
                            nc.sync.dma_start(
                                out[c * P:(c + 1) * P, h * N_SH:(h + 1) * N_SH],
                                ot)

    nc.compile()
    return nc


def _get_nc():
    if "nc" not in _CACHE:
        _CACHE["nc"] = _build()
    return _CACHE["nc"]


def _prep_shared(features_guests, Q, K, V):
    """Host-side layout prep shared by all cores (cast + transpose only)."""
    import ml_dtypes
    bf = ml_dtypes.bfloat16

    g = np.ascontiguousarray(np.asarray(features_guests, dtype=np.float32)).astype(bf)
    # gt[mo, p, do, j] = guests^T[do*128+p, mo*128+j] = g[mo*128+j, do*128+p]
    gt = np.ascontiguousarray(
        g.reshape(NMO, P, 8, P).transpose(0, 3, 2, 1))
    gn = g  # natural [8192, 1024]

    Qn = np.asarray(Q, dtype=np.float32)
    Kn = np.asarray(K, dtype=np.float32)
    Vn = np.asarray(V, dtype=np.float32)
    wq = np.ascontiguousarray(
        Qn.astype(bf).reshape(8, P, DIM).transpose(1, 0, 2))
    wkt = np.ascontiguousarray(
        Kn.T.astype(bf).reshape(8, P, DIM).transpose(1, 0, 2))
    wv = np.ascontiguousarray(
        Vn.astype(bf).reshape(8, P, DIM).transpose(1, 0, 2))
    return gt, gn, wq, wkt, wv


def make_in_maps(features_host, features_guests, Q, K, V):
    import ml_dtypes
    bf = ml_dtypes.bfloat16

    gt, gn, wq, wkt, wv = _prep_shared(features_guests, Q, K, V)
    fh = np.asarray(features_host, dtype=np.float32)

    in_maps = []
    for c in range(N_CORES):
        hs = fh[c * N_SH:(c + 1) * N_SH]           # [512, 1024]
        # ht[p, do, n] = hs[n, do*128+p]
        ht = np.ascontiguousarray(
            hs.T.astype(bf).reshape(8, P, N_SH).transpose(1, 0, 2))
        in_maps.append({
            "ht": ht, "gt": gt, "gn": gn,
            "wq": wq, "wkt": wkt, "wv": wv,
        })
    return in_maps


def kernel(features_host, features_guests, Q, K, V):
    from concourse.bass_utils import run_bass_kernel_spmd

    nc = _get_nc()
    in_maps = make_in_maps(features_host, features_guests, Q, K, V)
    res = run_bass_kernel_spmd(nc, in_maps, core_ids=list(range(N_CORES)))
    outs = [np.asarray(res.results[c]["out"]) for c in range(N_CORES)]
    return np.concatenate(outs, axis=0).astype(np.float32)


# revision 3
# speedup vs baseline: 1.7805x; 1.0362x over previous
"""Distributed attention kernel for 8 Trainium2 NeuronCores — zero-collective.

reference:
    query = features_host @ Q          # [4096, 1024]
    key   = features_guests @ K        # [8192, 1024]
    value = features_guests @ V        # [8192, 1024]
    att   = softmax(query @ key.T / 32, axis=1)
    out   = att @ value                # [4096, 1024]

Algebraic restructure so each core needs NO cross-core data:
    S   = query @ key^T = (host @ Q @ K^T) @ guests^T = q2 @ guests^T
    out = softmax(S) @ (guests @ V) = (P_norm @ guests) @ V = T_norm @ V
Host rows (N=4096) are sharded 512/core; guests (all 8192 rows) are
replicated to every core as bf16 in two layouts (transposed for the S
sweep, natural for the T sweep) prepared host-side. K folds into the
query side (q2 = host Q K^T, 2×1.07 GF) and V applies after guest
aggregation (O = T V, 1.07 GF), so the 16 MB keyT / 16 MB value
all-gathers of the collective formulation disappear entirely — along
with the ~100 us entry barrier and ~230 us of serial AG time.

Per-core pipeline (bf16 matmuls, fp32 PSUM accumulation, 20.4 GF):
  A: queryT = Q^T-chunks @ hostT; q2T = K @ queryT. DMAs and the
     accumulation loop are din-slice-granular so the PE starts ~1 us
     after the first 128-row slices of ht/wq land      (2.15 GF)
  B: S sweep over 64 m-chunks: S^T = guestsT_blk.T @ q2T, exp on
     ScalarE (scale=1/32) into persistent bf16 P [m, mo, n]; VectorE
     accumulates the rowsum as each chunk lands. The last S
     iterations also prefetch the first guests-natural chunks and wv
     so phase C starts without a DMA bubble            (8.6 GF)
  C: T^T sweep: T^T[din, n] += guests_blk.T @ P[:, mo, :] into all 8
     PSUM banks across the 64 m-chunks; gpsimd partition_all_reduce +
     reciprocal of the rowsum run here, off the PE path (8.6 GF)
  D: normalize T^T by the rowsum row during the PSUM->SBUF copy
     (VectorE tensor_mul, one per din chunk); O = T_norm^T @ V with
     the din loop innermost so each of the 8 output banks completes
     early and its copy-out + DRAM write overlap the rest (1.07 GF)
"""

import sys

for _p in ("/opt/trn_rl_repo", "/root/.axon_site/_ro/trn_rl_repo"):
    if _p not in sys.path:
        sys.path.insert(0, _p)

import numpy as np

N_HOST = 4096
N_GUEST = 8192
DIM = 1024
N_CORES = 8
N_SH = N_HOST // N_CORES      # 512 host rows per core
P = 128
NMO = N_GUEST // P            # 64 m-chunks of 128

_CACHE = {}


def _build():
    import concourse.bass as bass  # noqa: F401
    import concourse.mybir as mybir
    import concourse.tile as tile
    from concourse import bacc
    import concourse.bass_isa as bass_isa

    f32 = mybir.dt.float32
    bf16 = mybir.dt.bfloat16
    AF = mybir.ActivationFunctionType

    nc = bacc.Bacc(
        "TRN2",
        target_bir_lowering=False,
        debug=False,
        num_devices=N_CORES,
    )

    # host-prepped layouts (see kernel()):
    #   ht  [128, 8, 512]  = host_slice^T as [din_i, din_o, n]
    #   gt  [64, 128, 8, 128] = guests^T chunks [mo][din_i, din_o, m]
    #   gn  [8192, 1024]   = guests natural (bf16)
    #   wq  [128, 8, 1024] = Q as [din_i, din_o, dout]
    #   wkt [128, 8, 1024] = K^T as [dout_i, dout_o, din]
    #   wv  [128, 8, 1024] = V as [din_i, din_o, dout]
    ht = nc.dram_tensor("ht", [P, 8, N_SH], bf16, kind="ExternalInput").ap()
    gt = nc.dram_tensor("gt", [NMO, P, 8, P], bf16, kind="ExternalInput").ap()
    gn = nc.dram_tensor("gn", [N_GUEST, DIM], bf16, kind="ExternalInput").ap()
    wq = nc.dram_tensor("wq", [P, 8, DIM], bf16, kind="ExternalInput").ap()
    wkt = nc.dram_tensor("wkt", [P, 8, DIM], bf16, kind="ExternalInput").ap()
    wv = nc.dram_tensor("wv", [P, 8, DIM], bf16, kind="ExternalInput").ap()
    out = nc.dram_tensor("out", [N_SH, DIM], f32, kind="ExternalOutput").ap()

    with tile.TileContext(nc) as tc:
        with tc.tile_pool(name="persist", bufs=1) as persist:
            Psb = persist.tile([P, NMO, N_SH], bf16, name="Psb")      # 64KB/part
            qryT = persist.tile([P, 8, N_SH], bf16, name="qryT")
            q2T = persist.tile([P, 8, N_SH], bf16, name="q2T")
            ht_sb = persist.tile([P, 8, N_SH], bf16, name="ht_sb")
            wq_sb = persist.tile([P, 8, DIM], bf16, name="wq_sb")
            wkt_sb = persist.tile([P, 8, DIM], bf16, name="wkt_sb")
            wv_sb = persist.tile([P, 8, DIM], bf16, name="wv_sb")
            tTb = persist.tile([P, 8, N_SH], bf16, name="tTb")
            rs_acc = persist.tile([P, N_SH], f32, name="rs_acc")
            rs_red = persist.tile([P, N_SH], f32, name="rs_red")
            recip_row = persist.tile([P, N_SH], f32, name="recip_row")

            # ---- phase A: loads + query/q2 projections (din-sliced) ----
            with tc.tile_pool(name="ps_a", bufs=8, space="PSUM") as ps_a:
                # interleave ht/wq DMA slices so do-group 0 lands first
                for do in range(8):
                    nc.sync.dma_start(ht_sb[:, do, :], ht[:, do, :])
                    nc.sync.dma_start(wq_sb[:, do, :], wq[:, do, :])
                for do in range(8):
                    nc.sync.dma_start(wkt_sb[:, do, :], wkt[:, do, :])

                qp = [ps_a.tile([P, N_SH], f32, name=f"qp{dc}", tag="pa")
                      for dc in range(8)]
                for do in range(8):
                    for dc in range(8):
                        nc.tensor.matmul(
                            qp[dc],
                            lhsT=wq_sb[:, do, dc * P:(dc + 1) * P],
                            rhs=ht_sb[:, do, :],
                            start=(do == 0), stop=(do == 7),
                        )
                for dc in range(8):
                    nc.scalar.copy(out=qryT[:, dc, :], in_=qp[dc])

                q2p = [ps_a.tile([P, N_SH], f32, name=f"q2p{dc}", tag="pa")
                       for dc in range(8)]
                for do in range(8):
                    for dc in range(8):
                        nc.tensor.matmul(
                            q2p[dc],
                            lhsT=wkt_sb[:, do, dc * P:(dc + 1) * P],
                            rhs=qryT[:, do, :],
                            start=(do == 0), stop=(do == 7),
                        )
                for dc in range(8):
                    nc.scalar.copy(out=q2T[:, dc, :], in_=q2p[dc])

            # ---- phases B+C share the streaming pools ----
            with tc.tile_pool(name="gtp", bufs=1) as gtp, \
                 tc.tile_pool(name="gnp", bufs=1) as gnp:

                gn_tiles = [None] * NMO

                def issue_gn(k):
                    t_ = gnp.tile([P, DIM], bf16, name="gn_t",
                                  tag="gn", bufs=8)
                    nc.sync.dma_start(t_, gn[k * P:(k + 1) * P, :])
                    gn_tiles[k] = t_

                # ---- phase B: S sweep (S^T, exp, rowsum) ----
                with tc.tile_pool(name="ps_st", bufs=3, space="PSUM") as ps_st:
                    for mo in range(NMO):
                        gt_t = gtp.tile([P, 8, P], bf16, name="gt_t",
                                        tag="gt", bufs=8)
                        nc.sync.dma_start(gt_t, gt[mo])
                        if mo == 40:
                            # wv is first needed in phase D
                            nc.sync.dma_start(wv_sb, wv)
                        if mo >= 56:
                            issue_gn(mo - 56)
                        st = ps_st.tile([P, N_SH], f32, name="st", tag="st")
                        for do in range(8):
                            nc.tensor.matmul(
                                st,
                                lhsT=gt_t[:, do, :],
                                rhs=q2T[:, do, :],
                                start=(do == 0), stop=(do == 7),
                            )
                        nc.scalar.activation(
                            Psb[:, mo, :], st, AF.Exp, scale=1.0 / 32.0)
                        if mo == 0:
                            nc.vector.tensor_copy(out=rs_acc, in_=Psb[:, mo, :])
                        else:
                            nc.vector.tensor_add(
                                out=rs_acc, in0=rs_acc, in1=Psb[:, mo, :])

                # rowsum collapse across partitions + reciprocal: runs on
                # gpsimd/DVE during the T sweep, off the PE critical path
                nc.gpsimd.partition_all_reduce(
                    rs_red, rs_acc, P, bass_isa.ReduceOp.add)
                nc.vector.reciprocal(recip_row, rs_red)

                # ---- phase C: T^T sweep + normalize ----
                with tc.tile_pool(name="ps_t", bufs=8, space="PSUM") as ps_t:
                    tt = [ps_t.tile([P, N_SH], f32, name=f"tt{dc}", tag="tt")
                          for dc in range(8)]
                    for mo in range(NMO):
                        if mo + 8 < NMO:
                            issue_gn(mo + 8)
                        gn_t = gn_tiles[mo]
                        for dc in range(8):
                            nc.tensor.matmul(
                                tt[dc],
                                lhsT=gn_t[:, dc * P:(dc + 1) * P],
                                rhs=Psb[:, mo, :],
                                start=(mo == 0), stop=(mo == NMO - 1),
                            )
                    # normalize during the PSUM->SBUF copy (bf16 out)
                    for dc in range(8):
                        nc.vector.tensor_mul(
                            out=tTb[:, dc, :], in0=tt[dc], in1=recip_row)

                # ---- phase D: O = T_norm^T-chunks @ V, staggered out ----
                with tc.tile_pool(name="ps_o", bufs=4, space="PSUM") as ps_o, \
                     tc.tile_pool(name="outp", bufs=4) as outp:
                    for c in range(4):
                        for h in range(2):
                            o_t = ps_o.tile([P, N_SH], f32, name="o_t",
                                            tag="o")
                            for dc in range(8):
                                nc.tensor.matmul(
                                    o_t,
                                    lhsT=tTb[:, dc, c * P:(c + 1) * P],
                                    rhs=wv_sb[:, dc, h * N_SH:(h + 1) * N_SH],
                                    start=(dc == 0), stop=(dc == 7),
                                )
                            ot = outp.tile([P, N_SH], f32, name="ot", tag="ot")
                            # split tail copies across ACT and DVE
                            if h == 0:
                                nc.scalar.copy(out=ot, in_=o_t)
                            else:
                                nc.vector.tensor_copy(out=ot, in_=o_t)
                            nc.sync.dma_start(
                                out[c * P:(c + 1) * P, h * N_SH:(h + 1) * N_SH],
                                ot)

    nc.compile()
    return nc


def _get_nc():
    if "nc" not in _CACHE:
        _CACHE["nc"] = _build()
    return _CACHE["nc"]


def _prep_shared(features_guests, Q, K, V):
    """Host-side layout prep shared by all cores (cast + transpose only)."""
    import ml_dtypes
    bf = ml_dtypes.bfloat16

    g = np.ascontiguousarray(np.asarray(features_guests, dtype=np.float32)).astype(bf)
    # gt[mo, p, do, j] = guests^T[do*128+p, mo*128+j] = g[mo*128+j, do*128+p]
    gt = np.ascontiguousarray(
        g.reshape(NMO, P, 8, P).transpose(0, 3, 2, 1))
    gn = g  # natural [8192, 1024]

    Qn = np.asarray(Q, dtype=np.float32)
    Kn = np.asarray(K, dtype=np.float32)
    Vn = np.asarray(V, dtype=np.float32)
    wq = np.ascontiguousarray(
        Qn.astype(bf).reshape(8, P, DIM).transpose(1, 0, 2))
    wkt = np.ascontiguousarray(
        Kn.T.astype(bf).reshape(8, P, DIM).transpose(1, 0, 2))
    wv = np.ascontiguousarray(
        Vn.astype(bf).reshape(8, P, DIM).transpose(1, 0, 2))
    return gt, gn, wq, wkt, wv


def make_in_maps(features_host, features_guests, Q, K, V):
    import ml_dtypes
    bf = ml_dtypes.bfloat16

    gt, gn, wq, wkt, wv = _prep_shared(features_guests, Q, K, V)
    fh = np.asarray(features_host, dtype=np.float32)

    in_maps = []
    for c in range(N_CORES):
        hs = fh[c * N_SH:(c + 1) * N_SH]           # [512, 1024]
        # ht[p, do, n] = hs[n, do*128+p]
        ht = np.ascontiguousarray(
            hs.T.astype(bf).reshape(8, P, N_SH).transpose(1, 0, 2))
        in_maps.append({
            "ht": ht, "gt": gt, "gn": gn,
            "wq": wq, "wkt": wkt, "wv": wv,
        })
    return in_maps


def kernel(features_host, features_guests, Q, K, V):
    from concourse.bass_utils import run_bass_kernel_spmd

    nc = _get_nc()
    in_maps = make_in_maps(features_host, features_guests, Q, K, V)
    res = run_bass_kernel_spmd(nc, in_maps, core_ids=list(range(N_CORES)))
    outs = [np.asarray(res.results[c]["out"]) for c in range(N_CORES)]
    return np.concatenate(outs, axis=0).astype(np.float32)


# revision 5
# speedup vs baseline: 1.8410x; 1.0340x over previous
"""Distributed attention kernel for 8 Trainium2 NeuronCores — zero-collective.

reference:
    query = features_host @ Q          # [4096, 1024]
    key   = features_guests @ K        # [8192, 1024]
    value = features_guests @ V        # [8192, 1024]
    att   = softmax(query @ key.T / 32, axis=1)
    out   = att @ value                # [4096, 1024]

Algebraic restructure so each core needs NO cross-core data:
    S   = query @ key^T = (host @ Q @ K^T) @ guests^T = q2 @ guests^T
    out = softmax(S) @ (guests @ V) = (P @ guests) @ V / rowsum = T @ V / rs
Host rows (N=4096) are sharded 512/core; guests (all 8192 rows) are
replicated to every core as bf16 in two layouts (transposed for the S
sweep, natural for the T sweep) prepared host-side. K folds into the
query side (q2 = host Q K^T, 2×1.07 GF) and V applies after guest
aggregation (O = T V, 1.07 GF), so the 16 MB keyT / 16 MB value
all-gathers of the collective formulation disappear entirely — along
with the ~100 us entry barrier and ~230 us of serial AG time.

Per-core pipeline (bf16 matmuls, fp32 PSUM accumulation, 20.4 GF):
  A: queryT = Q^T-chunks @ hostT; q2T = K @ queryT. DMAs and the
     accumulation loop are din-slice-granular so the PE starts ~1 us
     after the first 128-row slices of ht/wq land. All PSUM->SBUF
     copy chains alternate ScalarE/VectorE to halve their exposure
     at phase boundaries (PSUM groups hoist deps on the full chain).
  B: S sweep over 64 m-chunks: S^T = guestsT_blk.T @ q2T, exp on
     ScalarE (scale=1/32) into persistent bf16 P [m, mo, n]; VectorE
     accumulates the rowsum as each chunk lands. The last S
     iterations also prefetch the first guests-natural chunks and wv.
  C: T^T sweep: T^T[din, n] += guests_blk.T @ P[:, mo, :] into all 8
     PSUM banks across the 64 m-chunks. Off the PE path, gpsimd
     partition_all_reduce collapses the rowsum, DVE 32x32 block
     transposes of the (replicated) rowsum row extract the
     per-partition layout rsT[q] = rowsum[c*128+q] from the diagonal
     blocks, and a reciprocal gives per-n-chunk [128,1] scalars.
  D: copy T^T to SBUF bf16 (ScalarE/VectorE alternating, trailing the
     per-bank accumulation stops); O = T^T-chunks @ V with the din
     loop innermost; the final PSUM->SBUF copy applies the softmax
     division as a per-partition tensor_scalar multiply, then DMA out.
"""

import sys

for _p in ("/opt/trn_rl_repo", "/root/.axon_site/_ro/trn_rl_repo"):
    if _p not in sys.path:
        sys.path.insert(0, _p)

import numpy as np

N_HOST = 4096
N_GUEST = 8192
DIM = 1024
N_CORES = 8
N_SH = N_HOST // N_CORES      # 512 host rows per core
P = 128
NMO = N_GUEST // P            # 64 m-chunks of 128

_CACHE = {}


def _build():
    import concourse.bass as bass  # noqa: F401
    import concourse.mybir as mybir
    import concourse.tile as tile
    from concourse import bacc
    import concourse.bass_isa as bass_isa

    f32 = mybir.dt.float32
    bf16 = mybir.dt.bfloat16
    AF = mybir.ActivationFunctionType

    nc = bacc.Bacc(
        "TRN2",
        target_bir_lowering=False,
        debug=False,
        num_devices=N_CORES,
    )

    # host-prepped layouts (see kernel()):
    #   ht  [128, 8, 512]  = host_slice^T as [din_i, din_o, n]
    #   gt  [64, 128, 8, 128] = guests^T chunks [mo][din_i, din_o, m]
    #   gn  [8192, 1024]   = guests natural (bf16)
    #   wq  [128, 8, 1024] = Q as [din_i, din_o, dout]
    #   wkt [128, 8, 1024] = K^T as [dout_i, dout_o, din]
    #   wv  [128, 8, 1024] = V as [din_i, din_o, dout]
    ht = nc.dram_tensor("ht", [P, 8, N_SH], bf16, kind="ExternalInput").ap()
    gt = nc.dram_tensor("gt", [NMO, P, 8, P], bf16, kind="ExternalInput").ap()
    gn = nc.dram_tensor("gn", [N_GUEST, DIM], bf16, kind="ExternalInput").ap()
    wq = nc.dram_tensor("wq", [P, 8, DIM], bf16, kind="ExternalInput").ap()
    wkt = nc.dram_tensor("wkt", [P, 8, DIM], bf16, kind="ExternalInput").ap()
    wv = nc.dram_tensor("wv", [P, 8, DIM], bf16, kind="ExternalInput").ap()
    out = nc.dram_tensor("out", [N_SH, DIM], f32, kind="ExternalOutput").ap()

    def psum_to_sbuf(idx, dst, src):
        # alternate engines so copy chains run two-wide
        if idx % 2 == 0:
            nc.scalar.copy(out=dst, in_=src)
        else:
            nc.vector.tensor_copy(out=dst, in_=src)

    with tile.TileContext(nc) as tc:
        with tc.tile_pool(name="persist", bufs=1) as persist:
            Psb = persist.tile([P, NMO, N_SH], bf16, name="Psb")      # 64KB/part
            qryT = persist.tile([P, 8, N_SH], bf16, name="qryT")
            q2Ts = [persist.tile([P, N_SH], bf16, name=f"q2T{i}")
                    for i in range(8)]
            ht_sb = persist.tile([P, 8, N_SH], bf16, name="ht_sb")
            wq_sb = persist.tile([P, 8, DIM], bf16, name="wq_sb")
            wkt_sb = persist.tile([P, 8, DIM], bf16, name="wkt_sb")
            wv_sb = persist.tile([P, 8, DIM], bf16, name="wv_sb")
            tTbs = [persist.tile([P, N_SH], bf16, name=f"tTb{i}")
                    for i in range(8)]
            rs_acc = persist.tile([P, N_SH], f32, name="rs_acc")
            rs_red = persist.tile([P, N_SH], f32, name="rs_red")
            rs_tr = persist.tile([P, P], f32, name="rs_tr")
            rsT4 = persist.tile([P, 4], f32, name="rsT4")
            recip4 = persist.tile([P, 4], f32, name="recip4")

            # ---- phase A: loads + query/q2 projections (din-sliced) ----
            with tc.tile_pool(name="ps_a", bufs=8, space="PSUM") as ps_a:
                # interleave ht/wq DMA slices so do-group 0 lands first
                for do in range(8):
                    nc.sync.dma_start(ht_sb[:, do, :], ht[:, do, :])
                    nc.sync.dma_start(wq_sb[:, do, :], wq[:, do, :])
                for do in range(8):
                    nc.sync.dma_start(wkt_sb[:, do, :], wkt[:, do, :])

                qp = [ps_a.tile([P, N_SH], f32, name=f"qp{dc}", tag="pa")
                      for dc in range(8)]
                for do in range(8):
                    for dc in range(8):
                        nc.tensor.matmul(
                            qp[dc],
                            lhsT=wq_sb[:, do, dc * P:(dc + 1) * P],
                            rhs=ht_sb[:, do, :],
                            start=(do == 0), stop=(do == 7),
                        )
                for dc in range(8):
                    psum_to_sbuf(dc, qryT[:, dc, :], qp[dc])

                q2p = [ps_a.tile([P, N_SH], f32, name=f"q2p{dc}", tag="pa")
                       for dc in range(8)]
                for do in range(8):
                    for dc in range(8):
                        nc.tensor.matmul(
                            q2p[dc],
                            lhsT=wkt_sb[:, do, dc * P:(dc + 1) * P],
                            rhs=qryT[:, do, :],
                            start=(do == 0), stop=(do == 7),
                        )
                for dc in range(8):
                    psum_to_sbuf(dc, q2Ts[dc], q2p[dc])

            # ---- phases B+C share the streaming pools ----
            with tc.tile_pool(name="gtp", bufs=1) as gtp, \
                 tc.tile_pool(name="gnp", bufs=1) as gnp:

                gn_tiles = [None] * NMO

                def issue_gn(k):
                    t_ = gnp.tile([P, DIM], bf16, name="gn_t",
                                  tag="gn", bufs=8)
                    nc.sync.dma_start(t_, gn[k * P:(k + 1) * P, :])
                    gn_tiles[k] = t_

                # ---- phase B: S sweep (S^T, exp, rowsum) ----
                with tc.tile_pool(name="ps_st", bufs=3, space="PSUM") as ps_st:
                    for mo in range(NMO):
                        gt_t = gtp.tile([P, 8, P], bf16, name="gt_t",
                                        tag="gt", bufs=8)
                        nc.sync.dma_start(gt_t, gt[mo])
                        if mo == 40:
                            # wv is first needed in phase D
                            nc.sync.dma_start(wv_sb, wv)
                        if mo >= 56:
                            issue_gn(mo - 56)
                        st = ps_st.tile([P, N_SH], f32, name="st", tag="st")
                        for do in range(8):
                            nc.tensor.matmul(
                                st,
                                lhsT=gt_t[:, do, :],
                                rhs=q2Ts[do],
                                start=(do == 0), stop=(do == 7),
                            )
                        nc.scalar.activation(
                            Psb[:, mo, :], st, AF.Exp, scale=1.0 / 32.0)
                        if mo == 0:
                            nc.vector.tensor_copy(out=rs_acc, in_=Psb[:, mo, :])
                        else:
                            nc.vector.tensor_add(
                                out=rs_acc, in0=rs_acc, in1=Psb[:, mo, :])

                # rowsum -> per-partition [128, 1] reciprocal scalars, all on
                # gpsimd/DVE during the T sweep, off the PE critical path:
                # partition_all_reduce leaves the full rowsum row in every
                # partition; block-transposing a row-replicated [128,128]
                # chunk makes its diagonal 32x32 blocks column-replicated,
                # so rsT4[q, c] = rowsum[c*128+q] via 4 tiny copies each.
                nc.gpsimd.partition_all_reduce(
                    rs_red, rs_acc, P, bass_isa.ReduceOp.add)
                for c in range(4):
                    nc.vector.transpose(
                        out=rs_tr, in_=rs_red[:, c * P:(c + 1) * P])
                    for j in range(4):
                        nc.vector.tensor_copy(
                            out=rsT4[32 * j:32 * (j + 1), c:c + 1],
                            in_=rs_tr[32 * j:32 * (j + 1), 32 * j:32 * j + 1])
                nc.vector.reciprocal(recip4, rsT4)

                # ---- phase C: T^T sweep ----
                with tc.tile_pool(name="ps_t", bufs=8, space="PSUM") as ps_t:
                    tt = [ps_t.tile([P, N_SH], f32, name=f"tt{dc}", tag="tt")
                          for dc in range(8)]
                    for mo in range(NMO):
                        if mo + 8 < NMO:
                            issue_gn(mo + 8)
                        gn_t = gn_tiles[mo]
                        for dc in range(8):
                            nc.tensor.matmul(
                                tt[dc],
                                lhsT=gn_t[:, dc * P:(dc + 1) * P],
                                rhs=Psb[:, mo, :],
                                start=(mo == 0), stop=(mo == NMO - 1),
                            )
                    # PSUM->SBUF copies trail the per-bank accumulation stops
                    for dc in range(8):
                        psum_to_sbuf(dc, tTbs[dc], tt[dc])

                # ---- phase D: O = T^T-chunks @ V, divide on copy-out ----
                with tc.tile_pool(name="ps_o", bufs=4, space="PSUM") as ps_o, \
                     tc.tile_pool(name="outp", bufs=4) as outp:
                    for c in range(4):
                        for h in range(2):
                            o_t = ps_o.tile([P, N_SH], f32, name="o_t",
                                            tag="o")
                            for dc in range(8):
                                nc.tensor.matmul(
                                    o_t,
                                    lhsT=tTbs[dc][:, c * P:(c + 1) * P],
                                    rhs=wv_sb[:, dc, h * N_SH:(h + 1) * N_SH],
                                    start=(dc == 0), stop=(dc == 7),
                                )
                            ot = outp.tile([P, N_SH], f32, name="ot", tag="ot")
                            # softmax divide fused into the tail copy,
                            # split across ACT and DVE
                            if h == 0:
                                nc.scalar.mul(ot, o_t, recip4[:, c:c + 1])
                            else:
                                nc.vector.tensor_scalar_mul(
                                    ot, o_t, recip4[:, c:c + 1])
                            nc.sync.dma_start(
                                out[c * P:(c + 1) * P, h * N_SH:(h + 1) * N_SH],
                                ot)

    nc.compile()
    return nc


def _get_nc():
    if "nc" not in _CACHE:
        _CACHE["nc"] = _build()
    return _CACHE["nc"]


def _prep_shared(features_guests, Q, K, V):
    """Host-side layout prep shared by all cores (cast + transpose only)."""
    import ml_dtypes
    bf = ml_dtypes.bfloat16

    g = np.ascontiguousarray(np.asarray(features_guests, dtype=np.float32)).astype(bf)
    # gt[mo, p, do, j] = guests^T[do*128+p, mo*128+j] = g[mo*128+j, do*128+p]
    gt = np.ascontiguousarray(
        g.reshape(NMO, P, 8, P).transpose(0, 3, 2, 1))
    gn = g  # natural [8192, 1024]

    Qn = np.asarray(Q, dtype=np.float32)
    Kn = np.asarray(K, dtype=np.float32)
    Vn = np.asarray(V, dtype=np.float32)
    wq = np.ascontiguousarray(
        Qn.astype(bf).reshape(8, P, DIM).transpose(1, 0, 2))
    wkt = np.ascontiguousarray(
        Kn.T.astype(bf).reshape(8, P, DIM).transpose(1, 0, 2))
    wv = np.ascontiguousarray(
        Vn.astype(bf).reshape(8, P, DIM).transpose(1, 0, 2))
    return gt, gn, wq, wkt, wv


def make_in_maps(features_host, features_guests, Q, K, V):
    import ml_dtypes
    bf = ml_dtypes.bfloat16

    gt, gn, wq, wkt, wv = _prep_shared(features_guests, Q, K, V)
    fh = np.asarray(features_host, dtype=np.float32)

    in_maps = []
    for c in range(N_CORES):
        hs = fh[c * N_SH:(c + 1) * N_SH]           # [512, 1024]
        # ht[p, do, n] = hs[n, do*128+p]
        ht = np.ascontiguousarray(
            hs.T.astype(bf).reshape(8, P, N_SH).transpose(1, 0, 2))
        in_maps.append({
            "ht": ht, "gt": gt, "gn": gn,
            "wq": wq, "wkt": wkt, "wv": wv,
        })
    return in_maps


def kernel(features_host, features_guests, Q, K, V):
    from concourse.bass_utils import run_bass_kernel_spmd

    nc = _get_nc()
    in_maps = make_in_maps(features_host, features_guests, Q, K, V)
    res = run_bass_kernel_spmd(nc, in_maps, core_ids=list(range(N_CORES)))
    outs = [np.asarray(res.results[c]["out"]) for c in range(N_CORES)]
    return np.concatenate(outs, axis=0).astype(np.float32)
